# revision 1
# baseline (speedup 1.0000x reference)
"""Bahdanau additive attention on 8 TRN2 NeuronCores.

Reference:
    Wx = enc @ W_a                        # [B,Te,H]
    Uh = dec @ U_a                        # [B,Td,H]
    scores[b,d,t] = sum_h V[h] * tanh(Wx[b,t,h] + Uh[b,d,h])
    e = softmax(scores, axis=t)           # [B,Td,Te]
    c = e @ enc                           # [B,Td,H]

Sharding: data-parallel over batch, 2 batches per core, no collectives.
Everything stays on-chip per batch; the [Td,Te,H] tanh intermediate is
produced in SBUF group-by-group and immediately contracted against V on
the tensor engine via a shifted-window masked-V stationary operand, so
scores land directly in a [Td,Te] PSUM bank.
"""

import numpy as np

B, TE, TD, H = 16, 512, 128, 128
NCORES = 8
BPC = B // NCORES  # batches per core
TEC = TE // 128    # 4 column chunks of Te
G = 16             # decoder positions per tanh group
NGRP = TD // G

_CACHE = {}


def _build_nc():
    import concourse.mybir as mybir
    import concourse.tile as tile
    from concourse import bacc
    from concourse.masks import make_identity

    FP32 = mybir.dt.float32
    AX = mybir.AxisListType.X
    AF = mybir.ActivationFunctionType

    nc = bacc.Bacc("TRN2", target_bir_lowering=False)

    enc_d = nc.dram_tensor("enc", [BPC, TE, H], FP32, kind="ExternalInput")
    dec_d = nc.dram_tensor("dec", [BPC, TD, H], FP32, kind="ExternalInput")
    wa_d = nc.dram_tensor("w_a", [H, H], FP32, kind="ExternalInput")
    ua_d = nc.dram_tensor("u_a", [H, H], FP32, kind="ExternalInput")
    va_d = nc.dram_tensor("v_a", [H, 1], FP32, kind="ExternalInput")
    c_d = nc.dram_tensor("c_out", [BPC, TD, H], FP32, kind="ExternalOutput")
    e_d = nc.dram_tensor("e_out", [BPC, TD, TE], FP32, kind="ExternalOutput")

    with tile.TileContext(nc) as tc:
        with (
            tc.tile_pool(name="const", bufs=1) as constp,
            tc.tile_pool(name="prep", bufs=2) as prep,
            tc.tile_pool(name="big", bufs=2) as bigp,
            tc.tile_pool(name="soft", bufs=2) as softp,
            tc.tile_pool(name="ps_scores", bufs=2, space="PSUM") as ps_scores,
            tc.tile_pool(name="ps_wide", bufs=1, space="PSUM") as ps_wide,
            tc.tile_pool(name="ps_tr", bufs=2, space="PSUM") as ps_tr,
            tc.tile_pool(name="ps_c", bufs=2, space="PSUM") as ps_c,
        ):
            wa = constp.tile([H, H], FP32)
            nc.sync.dma_start(wa[:], wa_d[:])
            ua = constp.tile([H, H], FP32)
            nc.sync.dma_start(ua[:], ua_d[:])
            # zv: zeros except column 128 = V. lhsT window zv[:, 128-d:256-d]
            # has V in its column d and zeros elsewhere, so the matmul adds
            # score_d into partition row d of the scores bank.
            zv = constp.tile([H, 257], FP32)
            nc.vector.memset(zv[:], 0.0)
            nc.sync.dma_start(zv[:, 128:129], va_d[:])
            ident = constp.tile([128, 128], FP32)
            make_identity(nc, ident[:])

            for b in range(BPC):
                enc = prep.tile([128, TEC, 128], FP32, tag="enc")
                nc.sync.dma_start(
                    enc[:], enc_d[b].rearrange("(c p) f -> p c f", p=128)
                )
                dec = prep.tile([TD, H], FP32, tag="dec")
                nc.sync.dma_start(dec[:], dec_d[b])

                # enc^T [H, Te], dec^T [H, Td]
                enc_t = prep.tile([H, TE], FP32, tag="enc_t")
                for c in range(TEC):
                    pt = ps_tr.tile([128, 128], FP32, tag="pt")
                    nc.tensor.transpose(pt[:], enc[:, c, :], ident[:])
                    nc.vector.tensor_copy(enc_t[:, c * 128 : (c + 1) * 128], pt[:])
                dec_t = prep.tile([H, TD], FP32, tag="dec_t")
                ptd = ps_tr.tile([128, 128], FP32, tag="pt")
                nc.tensor.transpose(ptd[:], dec[:], ident[:])
                nc.vector.tensor_copy(dec_t[:], ptd[:])

                # Wx^T[k,t] = sum_h W_a[h,k] enc^T[h,t]
                wx_t = prep.tile([H, TE], FP32, tag="wx_t")
                pw = ps_wide.tile([H, TE], FP32, tag="pw")
                nc.tensor.matmul(pw[:], wa[:], enc_t[:], start=True, stop=True)
                nc.scalar.copy(wx_t[:], pw[:])
                # Uh^T[k,d] = sum_h U_a[h,k] dec^T[h,d]
                uh_t = prep.tile([H, TD], FP32, tag="uh_t")
                pu = ps_tr.tile([128, 128], FP32, tag="pt")
                nc.tensor.matmul(pu[:], ua[:], dec_t[:], start=True, stop=True)
                nc.vector.tensor_copy(uh_t[:], pu[:])

                scores = ps_scores.tile([TD, TE], FP32, tag="scores")
                for g in range(NGRP):
                    addb = bigp.tile([128, G, TE], FP32, tag="addb")
                    for j in range(G):
                        d = g * G + j
                        nc.vector.tensor_scalar_add(
                            addb[:, j, :], wx_t[:], uh_t[:, d : d + 1]
                        )
                    tanhb = bigp.tile([128, G, TE], FP32, tag="tanhb")
                    nc.scalar.activation(tanhb[:], addb[:], AF.Tanh)
                    for j in range(G):
                        d = g * G + j
                        nc.tensor.matmul(
                            scores[:],
                            zv[:, 128 - d : 256 - d],
                            tanhb[:, j, :],
                            start=(d == 0),
                            stop=(d == TD - 1),
                        )

                # softmax along Te (free axis)
                mx = softp.tile([TD, 1], FP32, tag="mx")
                nc.vector.reduce_max(mx[:], scores[:], axis=AX)
                nmx = softp.tile([TD, 1], FP32, tag="nmx")
                nc.vector.tensor_scalar_mul(nmx[:], mx[:], -1.0)
                e_raw = softp.tile([TD, TE], FP32, tag="e_raw")
                ssum = softp.tile([TD, 1], FP32, tag="ssum")
                nc.scalar.activation(
                    e_raw[:],
                    scores[:],
                    AF.Exp,
                    bias=nmx[:, 0:1],
                    accum_out=ssum[:, 0:1],
                )
                rs = softp.tile([TD, 1], FP32, tag="rs")
                nc.vector.reciprocal(rs[:], ssum[:])
                e_n = softp.tile([TD, TE], FP32, tag="e_n")
                nc.vector.tensor_scalar_mul(e_n[:], e_raw[:], rs[:, 0:1])
                nc.sync.dma_start(e_d[b], e_n[:])

                # c = e @ enc : transpose e chunks then accumulate
                e_t = softp.tile([128, TEC, 128], FP32, tag="e_t")
                for c in range(TEC):
                    pt = ps_tr.tile([128, 128], FP32, tag="pt")
                    nc.tensor.transpose(
                        pt[:], e_n[:, c * 128 : (c + 1) * 128], ident[:]
                    )
                    nc.vector.tensor_copy(e_t[:, c, :], pt[:])
                pc = ps_c.tile([TD, H], FP32, tag="pc")
                for c in range(TEC):
                    nc.tensor.matmul(
                        pc[:],
                        e_t[:, c, :],
                        enc[:, c, :],
                        start=(c == 0),
                        stop=(c == TEC - 1),
                    )
                cout = softp.tile([TD, H], FP32, tag="cout")
                nc.vector.tensor_copy(cout[:], pc[:])
                nc.sync.dma_start(c_d[b], cout[:])

    nc.compile()
    return nc


def get_nc():
    if "nc" not in _CACHE:
        _CACHE["nc"] = _build_nc()
    return _CACHE["nc"]


def make_in_maps(encoder_out_seq, decoder_out_seq, W_a, U_a, V_a):
    enc = np.ascontiguousarray(np.asarray(encoder_out_seq, dtype=np.float32))
    dec = np.ascontiguousarray(np.asarray(decoder_out_seq, dtype=np.float32))
    wa = np.ascontiguousarray(np.asarray(W_a, dtype=np.float32))
    ua = np.ascontiguousarray(np.asarray(U_a, dtype=np.float32))
    va = np.ascontiguousarray(np.asarray(V_a, dtype=np.float32))
    return [
        {
            "enc": enc[i * BPC : (i + 1) * BPC],
            "dec": dec[i * BPC : (i + 1) * BPC],
            "w_a": wa,
            "u_a": ua,
            "v_a": va,
        }
        for i in range(NCORES)
    ]


def gather_outputs(results):
    c = np.concatenate([results[i]["c_out"] for i in range(NCORES)], axis=0)
    e = np.concatenate([results[i]["e_out"] for i in range(NCORES)], axis=0)
    return np.asarray(c, dtype=np.float32), np.asarray(e, dtype=np.float32)


def kernel(encoder_out_seq, decoder_out_seq, W_a, U_a, V_a):
    from concourse.bass_utils import run_bass_kernel_spmd

    nc = get_nc()
    in_maps = make_in_maps(encoder_out_seq, decoder_out_seq, W_a, U_a, V_a)
    res = run_bass_kernel_spmd(nc, in_maps, core_ids=list(range(NCORES)))
    return gather_outputs(res.results)
